# revision 33
# baseline (speedup 1.0000x reference)
"""Multi-head attention (B=2, N=2048, D=2048, 16 heads) on 8 NeuronCores.

Sharding: tensor-parallel over heads (2 heads/core) for QKV projections and
attention; one AllToAll per (head, batch) re-shards the attention context
from head-split to row-split; the output projection is row-parallel
(512 rows/core) with the full Wo on every core.

All matmul operands are bf16 (PSUM accumulation stays fp32): same 1 cycle/row
PE rate as fp32r but half the LDWEIGHTS time, DMA bytes, SBUF footprint and
AllToAll bytes. Measured rel err ~1e-3 against the fp32 reference (tolerance
2e-2).

Layout strategy (everything contracts on the SBUF partition axis):
  - host feeds xT = x.T so projections need no on-device transposes
  - Q, K are produced transposed ([head_dim, rows]); V in natural layout by
    swapping stationary/moving in its projection matmuls
  - scores are computed transposed: S.T[k_row, q_row] = (K.T)^T . Q.T chunks,
    two k-chunks into one 2-bank PSUM tile so a single ACT exp covers
    [128,1024] (amortizes the 352-cycle ACT fixed overhead)
  - softmax skips the max-subtraction (scores ~ N(0,1); fp32 exp is safe);
    the denominator rides an all-ones [128,128] stationary matmul so the
    row-sums arrive pre-broadcast across partitions in PSUM: reciprocal +
    multiply run straight off PSUM, no DRAM bounce
  - PV and denominator matmuls are software-pipelined one kc-pair behind the
    score matmuls (pipeline spans all quarters) so the PE never waits on ACT
  - v-bias and o-bias commute out of the kernel: attention rows sum to 1, so
    out = attn@(v0+bv)@Wo.T + bo = device_out + (Wo@bv + bo); host adds it.

One PSUM pool with shared tags serves all three phases (no mid-kernel pool
drains): stA [128,2x512] bufs=2 (4 banks) + ctx/cs [128,512] bufs=2
(2 banks each) = 8 banks. Full Wo (8MB bf16) prefetches on the otherwise
idle scalar DMA queue during phases 1-2; phase 3 runs batch-0 first so its
matmuls hide the last AllToAll.
"""

import numpy as np
import ml_dtypes

import concourse.bacc as bacc
import concourse.mybir as mybir
import concourse.tile as tile
from concourse.bass_utils import run_bass_kernel_spmd

P = 128          # partitions
B = 2            # batch
SEQ = 2048       # sequence length
D = 2048         # hidden
H = 16           # heads
HD = D // H      # head dim = 128
W = 8            # cores
HPC = H // W     # heads per core = 2
DPC = HPC * HD   # features per core = 256
RPC = B * SEQ // W   # rows per core after re-shard = 512
FC = D // P      # feature chunks = 16
RT = B * SEQ     # total rows = 4096
KRC = SEQ // P   # key-row chunks per batch = 16
KP = KRC // 2    # key-row chunk pairs = 8
QRC = SEQ // 512  # query chunks of 512 per batch = 4
HB = RPC // B    # rows per core per batch = 256

f32 = mybir.dt.float32
bf16 = mybir.dt.bfloat16

INV_SQRT_HD = 1.0 / float(np.sqrt(HD))
Act = mybir.ActivationFunctionType

_CACHED_NC = None


def build_nc(dbg=False):
    _pend.clear()
    nc = bacc.Bacc("TRN2", target_bir_lowering=False, debug=False)

    # x pre-shuffled on host so each [P, 512] tile is one contiguous DMA:
    # xP[fc, rc, p, c] = x.T[fc*128 + p, rc*512 + c]
    xP = nc.dram_tensor("xP", [FC, RT // 512, P, 512], bf16,
                        kind="ExternalInput")
    # projection weights pre-shuffled on host into SBUF layout
    # [p, fc, c] = W.T[fc*128 + p, c] so each loads as one contiguous DMA
    wqP = nc.dram_tensor("wqP", [P, FC, DPC], bf16, kind="ExternalInput")
    wkP = nc.dram_tensor("wkP", [P, FC, DPC], bf16, kind="ExternalInput")
    wvP = nc.dram_tensor("wvP", [P, FC, DPC], bf16, kind="ExternalInput")
    bq = nc.dram_tensor("bq", [P, HPC], f32, kind="ExternalInput")
    bk = nc.dram_tensor("bk", [P, HPC], f32, kind="ExternalInput")
    woT = nc.dram_tensor("woT", [D, D], bf16, kind="ExternalInput")
    # out rows: [0:256] = this core's batch-0 rows, [256:512] = batch-1 rows
    out = nc.dram_tensor("out", [RPC, D], f32, kind="ExternalOutput")
    if dbg:
        d_qT = nc.dram_tensor("d_qT", [P, HPC, RT], bf16, kind="ExternalOutput")
        d_kT = nc.dram_tensor("d_kT", [P, HPC, RT], bf16, kind="ExternalOutput")
        d_v = nc.dram_tensor("d_v", [P, RT // P, DPC], bf16,
                             kind="ExternalOutput")
        d_ctxl = nc.dram_tensor("d_ctxl", [HPC, B, P, W, HB], bf16,
                                kind="ExternalOutput")

    with tile.TileContext(nc) as tc:
        with (
            tc.tile_pool(name="persist", bufs=1) as persist,
            tc.tile_pool(name="dram", bufs=1, space="DRAM") as dram,
            tc.tile_pool(name="ps", bufs=2, space="PSUM") as ps,
            tc.tile_pool(name="wo", bufs=1) as wop,
            tc.tile_pool(name="xtp", bufs=12) as xtp,
            tc.tile_pool(name="wproj", bufs=1) as wproj,
            tc.tile_pool(name="attn_sb", bufs=4) as attn_sb,
            tc.tile_pool(name="norm_sb", bufs=2) as norm_sb,
            tc.tile_pool(name="ctxlp", bufs=1) as ctxlp,
            tc.tile_pool(name="osbp", bufs=2) as osbp,
        ):
            # ---- persistent SBUF state ----
            qT_sb = persist.tile([P, HPC, RT], bf16)      # [hd, h, row]
            kT_sb = persist.tile([P, HPC, RT], bf16)
            v_sb = persist.tile([P, RT // P, DPC], bf16)  # [row%128, rchunk, d]
            bq_sb = persist.tile([P, HPC], f32)
            bk_sb = persist.tile([P, HPC], f32)
            ones_sb = persist.tile([P, P], bf16)

            # one A2A per (head, batch): shard j = ctx.T for batch-b rows
            # [HB*j, HB*(j+1)) in head h's feature block
            a2a_in = [[dram.tile([W, HD, HB], bf16, name=f"a2a_in{h}{b}")
                       for b in range(B)] for h in range(HPC)]
            a2a_out = [[dram.tile([W, HD, HB], bf16, name=f"a2a_out{h}{b}")
                        for b in range(B)] for h in range(HPC)]

            nc.vector.memset(ones_sb[:], 1.0)
            nc.sync.dma_start(bq_sb[:], bq.ap())
            nc.sync.dma_start(bk_sb[:], bk.ap())

            # ---- HAM warmup: ~3us of tiny matmuls so the PE clock gate
            # opens before the real work arrives ----
            wtile = ps.tile([1, 4], f32, tag="cs", name="warm")
            for i in range(64):
                nc.tensor.matmul(wtile[:], ones_sb[:, 0:1],
                                 ones_sb[:, 0:4], start=True, stop=True)

            # ---- projection weights first, then full-Wo prefetch, all on
            # the scalar engine's DMA queue so the sync queue carries only
            # xt tiles during phase 1 ----
            wq_sb = wproj.tile([P, FC, DPC], bf16)
            wk_sb = wproj.tile([P, FC, DPC], bf16)
            wv_sb = wproj.tile([P, FC, DPC], bf16)
            # 4 chunks per weight so the first fc chunks land early
            for c4 in range(4):
                fsl = slice(4 * c4, 4 * c4 + 4)
                nc.scalar.dma_start(wq_sb[:, fsl, :], wqP.ap()[:, fsl, :])
                nc.scalar.dma_start(wk_sb[:, fsl, :], wkP.ap()[:, fsl, :])
                nc.scalar.dma_start(wv_sb[:, fsl, :], wvP.ap()[:, fsl, :])
            wo_tiles = {}

            def prefetch_wo(jc, hh):
                # 8 tiles per call, trickled through phase 1 on the idle
                # gpsimd queue so they never compete with phase-1 loads
                for i in range(W):
                    t = wop.tile([P, 512], bf16, name=f"wo_{jc}_{hh}_{i}")
                    nc.gpsimd.dma_start(
                        t[:],
                        woT.ap()[i * DPC + hh * HD:i * DPC + (hh + 1) * HD,
                                 jc * 512:(jc + 1) * 512])
                    wo_tiles[(jc, hh, i)] = t

            # ---- phase 1: QKV projections ----
            for rc in range(RT // 512):  # 8 row chunks of 512
                q_ps = ps.tile([P, 2, 512], f32, tag="stA", name="q_ps")
                k_ps = ps.tile([P, 2, 512], f32, tag="stA", name="k_ps")
                # one PSUM bank per V stream: matmul accumulation state is
                # bank-granular, so streams must not share a bank
                v_ps = [ps.tile([P, DPC], f32, tag=("ctx", "cs")[s // 2],
                                name=f"v_ps{s}")[:] for s in range(4)]
                for fc in range(FC):
                    xt = xtp.tile([P, 512], bf16, tag="xt")
                    nc.sync.dma_start(xt[:], xP.ap()[fc, rc])
                    st = fc == 0
                    sp = fc == FC - 1
                    # interleave short-stream V matmuls between long Q/K
                    # streams so each LDWEIGHTS hides behind a stream
                    for i in range(HPC):
                        nc.tensor.matmul(
                            q_ps[:, i, :], wq_sb[:, fc, i * HD:(i + 1) * HD],
                            xt[:], start=st, stop=sp)
                        nc.tensor.matmul(
                            v_ps[2 * i],
                            xt[:, 2 * i * P:(2 * i + 1) * P],
                            wv_sb[:, fc, :], start=st, stop=sp)
                        nc.tensor.matmul(
                            k_ps[:, i, :], wk_sb[:, fc, i * HD:(i + 1) * HD],
                            xt[:], start=st, stop=sp)
                        nc.tensor.matmul(
                            v_ps[2 * i + 1],
                            xt[:, (2 * i + 1) * P:(2 * i + 2) * P],
                            wv_sb[:, fc, :], start=st, stop=sp)
                # PSUM -> SBUF; Q/K on ACT (with bias), V on DVE
                for i in range(HPC):
                    nc.scalar.activation(
                        qT_sb[:, i, rc * 512:(rc + 1) * 512], q_ps[:, i, :],
                        Act.Identity, bias=bq_sb[:, i:i + 1])
                    nc.scalar.activation(
                        kT_sb[:, i, rc * 512:(rc + 1) * 512], k_ps[:, i, :],
                        Act.Identity, bias=bk_sb[:, i:i + 1])
                for s4 in range(4):
                    nc.vector.tensor_copy(v_sb[:, rc * 4 + s4, :], v_ps[s4])
                prefetch_wo(rc // 2, rc % 2)

            # ctxl staging tiles (consumed by phase 3); loaded right after
            # each quarter's collective on the gpsimd queue
            ctxl = [[ctxlp.tile([P, W, HB], bf16, name=f"ctxl{h}{b}")
                     for b in range(B)] for h in range(HPC)]

            # ---- phase 2: attention; one A2A per (h, b) quarter ----
            # PV + denominator matmuls run one kc-pair behind the score
            # matmuls so the PE never waits on the ACT exp; the pipeline
            # spans qc and quarter boundaries.
            prev = None

            def emit_pv(state):
                pt2, h, b, ctx_ps, cs_ps, kp = state
                for j in range(2):
                    kc = 2 * kp + j
                    st = kc == 0
                    sp = kc == KRC - 1
                    nc.tensor.matmul(
                        ctx_ps[:], v_sb[:, b * KRC + kc, h * HD:(h + 1) * HD],
                        pt2[:, j, :], start=st, stop=sp)
                    nc.tensor.matmul(
                        cs_ps[:], ones_sb[:], pt2[:, j, :], start=st, stop=sp)

            for h in range(HPC):
                for b in range(B):
                    for qc in range(QRC):
                        ctx_ps = ps.tile([P, 512], f32, tag="ctx",
                                         name="ctx_ps")
                        cs_ps = ps.tile([P, 512], f32, tag="cs", name="cs_ps")
                        for kp in range(KP):
                            st2 = ps.tile([P, 2, 512], f32, tag="stA",
                                          name="st2")
                            for j in range(2):
                                kc = 2 * kp + j
                                nc.tensor.matmul(
                                    st2[:, j, :],
                                    kT_sb[:, h, b * SEQ + kc * P:
                                          b * SEQ + (kc + 1) * P],
                                    qT_sb[:, h, b * SEQ + qc * 512:
                                          b * SEQ + (qc + 1) * 512],
                                    start=True, stop=True)
                            pt2 = attn_sb.tile([P, 2, 512], bf16, tag="pt")
                            nc.scalar.activation(pt2[:], st2[:], Act.Exp,
                                                 scale=INV_SQRT_HD)
                            if prev is not None:
                                emit_pv(prev)
                            prev = (pt2, h, b, ctx_ps, cs_ps, kp)
                        # normalize + ship the previous qc once its last
                        # PV/denominator matmuls have been emitted
                        if qc > 0:
                            _norm_ship(nc, norm_sb, a2a_in, *_pend.pop(0))
                        _pend.append((h, b, qc, ctx_ps, cs_ps))
                    # flush so this quarter's A2A can fire
                    emit_pv(prev)
                    prev = None
                    _norm_ship(nc, norm_sb, a2a_in, *_pend.pop(0))
                    nc.gpsimd.collective_compute(
                        "AllToAll", mybir.AluOpType.bypass,
                        replica_groups=[list(range(W))],
                        ins=[a2a_in[h][b][:]], outs=[a2a_out[h][b][:]])
                    for i in range(W):
                        nc.gpsimd.dma_start(
                            ctxl[h][b][:, i, :], a2a_out[h][b][i, :, :])

            if dbg:
                nc.sync.dma_start(d_qT.ap(), qT_sb[:])
                nc.sync.dma_start(d_kT.ap(), kT_sb[:])
                nc.sync.dma_start(d_v.ap(), v_sb[:])
                for hh in range(HPC):
                    for bb in range(B):
                        nc.sync.dma_start(d_ctxl.ap()[hh, bb],
                                          ctxl[hh][bb][:])

            # ---- phase 3: output projection ----
            # batch 0 first: its matmuls hide the last A2A (h=1, b=1)
            for bb in range(B):
                for jc in range(D // 512):
                    o_ps = [ps.tile([P, 512], f32, tag=("ctx", "cs")[r2],
                                    name=f"o_ps{r2}") for r2 in range(HB // P)]
                    for hh in range(HPC):
                        for i in range(W):
                            st = hh == 0 and i == 0
                            sp = hh == HPC - 1 and i == W - 1
                            for r2 in range(HB // P):
                                nc.tensor.matmul(
                                    o_ps[r2][:],
                                    ctxl[hh][bb][:, i, r2 * P:(r2 + 1) * P],
                                    wo_tiles[(jc, hh, i)][:],
                                    start=st, stop=sp)
                    for r2 in range(HB // P):
                        o_sb = osbp.tile([P, 512], f32, tag="osb")
                        nc.scalar.activation(o_sb[:], o_ps[r2][:], Act.Copy)
                        nc.sync.dma_start(
                            out.ap()[(bb * 2 + r2) * P:(bb * 2 + r2 + 1) * P,
                                     jc * 512:(jc + 1) * 512],
                            o_sb[:])

    nc.compile()
    return nc


_pend = []


def _norm_ship(nc, norm_sb, a2a_in, h, b, qc, ctx_ps, cs_ps):
    """Reciprocal of the pre-broadcast row sums, normalize ctx, ship to the
    A2A staging slots. Runs on DVE straight off PSUM."""
    rcp = norm_sb.tile([P, 512], f32, tag="rcp", name="rcp")
    nc.vector.reciprocal(rcp[:], cs_ps[:])
    ctxn = norm_sb.tile([P, 512], bf16, tag="ctxn", name="ctxn")
    nc.vector.tensor_mul(ctxn[:], ctx_ps[:], rcp[:])
    for s2 in range(2):
        nc.gpsimd.dma_start(a2a_in[h][b][2 * qc + s2, :, :],
                            ctxn[:, s2 * HB:(s2 + 1) * HB])


def kernel(x, Wq, bq, Wk, bk, Wv, bv, Wo, bo, _run_kwargs=None):
    global _CACHED_NC
    if _CACHED_NC is None:
        _CACHED_NC = build_nc()
    nc = _CACHED_NC

    bf = ml_dtypes.bfloat16
    x = np.asarray(x, dtype=np.float32)
    Wq = np.asarray(Wq, dtype=np.float32)
    Wk = np.asarray(Wk, dtype=np.float32)
    Wv = np.asarray(Wv, dtype=np.float32)
    Wo = np.asarray(Wo, dtype=np.float32)
    bq = np.asarray(bq, dtype=np.float32)
    bk = np.asarray(bk, dtype=np.float32)
    bv = np.asarray(bv, dtype=np.float32)
    bo = np.asarray(bo, dtype=np.float32)

    xT = x.reshape(RT, D).T.astype(bf)                         # [D, RT]
    xP = np.ascontiguousarray(
        xT.reshape(FC, P, RT // 512, 512).transpose(0, 2, 1, 3))
    woT = np.ascontiguousarray(Wo.T).astype(bf)                # [D, D]
    bo_eff = (bo + Wo @ bv).astype(np.float32)                 # [D]

    in_maps = []
    for i in range(W):
        sl = slice(i * DPC, (i + 1) * DPC)
        def shuf(Wm):
            # [P, FC, DPC]: [p, fc, c] = Wm[sl].T[fc*128 + p, c]
            return np.ascontiguousarray(
                Wm[sl, :].T.reshape(FC, P, DPC).transpose(1, 0, 2)).astype(bf)
        in_maps.append({
            "xP": xP,
            "wqP": shuf(Wq),
            "wkP": shuf(Wk),
            "wvP": shuf(Wv),
            "bq": np.ascontiguousarray(bq[sl].reshape(HPC, P).T),
            "bk": np.ascontiguousarray(bk[sl].reshape(HPC, P).T),
            "woT": woT,
        })

    kw = _run_kwargs or {}
    res = run_bass_kernel_spmd(nc, in_maps, core_ids=list(range(W)), **kw)

    full = np.empty((RT, D), dtype=np.float32)
    for i in range(W):
        o = res.results[i]["out"]
        full[i * HB:(i + 1) * HB, :] = o[:HB]              # batch 0 rows
        full[SEQ + i * HB:SEQ + (i + 1) * HB, :] = o[HB:]  # batch 1 rows
    full += bo_eff[None, :]
    out = full.reshape(B, SEQ, D)
    if kw:
        kernel.last_results = res
    return out


# revision 35
# speedup vs baseline: 1.0014x; 1.0014x over previous
"""Multi-head attention (B=2, N=2048, D=2048, 16 heads) on 8 NeuronCores.

Sharding: tensor-parallel over heads (2 heads/core) for QKV projections and
attention; one AllToAll per (head, batch) re-shards the attention context
from head-split to row-split; the output projection is row-parallel
(512 rows/core) with the full Wo on every core.

All matmul operands are bf16 (PSUM accumulation stays fp32): same 1 cycle/row
PE rate as fp32r but half the LDWEIGHTS time, DMA bytes, SBUF footprint and
AllToAll bytes. Measured rel err ~1e-3 against the fp32 reference (tolerance
2e-2).

Layout strategy (everything contracts on the SBUF partition axis):
  - host pre-shuffles x and the projection weights into the exact SBUF tile
    layouts so every phase-1 DMA is fully contiguous (bf16 halves the DMA
    line size; un-shuffled loads were descriptor-bound at kernel start)
  - Q, K are produced transposed ([head_dim, rows]); V in natural layout by
    swapping stationary/moving in its projection matmuls (one full PSUM bank
    per V stream: matmul accumulation state is bank-granular)
  - scores are computed transposed: S.T[k_row, q_row] = (K.T)^T . Q.T chunks,
    two k-chunks into one 2-bank PSUM tile so a single ACT exp covers
    [128,1024] (amortizes the 352-cycle ACT fixed overhead; unpaired exps
    make phase 2 ACT-bound)
  - softmax skips the max-subtraction (scores ~ N(0,1); fp32 exp is safe);
    the denominator rides an all-ones [128,128] stationary matmul so the
    row-sums arrive pre-broadcast across partitions in PSUM: reciprocal +
    multiply run straight off PSUM, no DRAM bounce
  - PV and denominator matmuls are software-pipelined one kc-pair behind the
    score matmuls (pipeline spans all quarters) so the PE never waits on ACT
  - v-bias and o-bias commute out of the kernel: attention rows sum to 1, so
    out = attn@(v0+bv)@Wo.T + bo = device_out + (Wo@bv + bo); host adds it.

One PSUM pool with shared tags serves all three phases (no mid-kernel pool
drains): stA [128,2x512] bufs=2 (4 banks) + ctx/cs [128,512] bufs=2
(2 banks each) = 8 banks. Full Wo (8MB bf16) prefetches on the otherwise
idle gpsimd DMA queue, trickled 8 tiles per phase-1 row chunk so it never
competes with phase-1 loads; phase 3 runs batch-0 first so its matmuls hide
the last AllToAll.

Rejected directions (measured/derived): fp8 DoubleRow for PV needs V in
fp8e4m3, which alone costs 2.5e-2 rel err (over the 2e-2 gate); scores
can't DoubleRow (contraction is a single 128 k-tile); GPSIMD elementwise
(~2.6 cyc/elem) is too slow to take over the denominator accumulation. The
PE runs at ~78% of peak under a board-level GPIO power throttle, which sets
the current floor.
"""

import numpy as np
import ml_dtypes

import concourse.bacc as bacc
import concourse.mybir as mybir
import concourse.tile as tile
from concourse.bass_utils import run_bass_kernel_spmd

P = 128          # partitions
B = 2            # batch
SEQ = 2048       # sequence length
D = 2048         # hidden
H = 16           # heads
HD = D // H      # head dim = 128
W = 8            # cores
HPC = H // W     # heads per core = 2
DPC = HPC * HD   # features per core = 256
RPC = B * SEQ // W   # rows per core after re-shard = 512
FC = D // P      # feature chunks = 16
RT = B * SEQ     # total rows = 4096
KRC = SEQ // P   # key-row chunks per batch = 16
KP = KRC // 2    # key-row chunk pairs = 8
QRC = SEQ // 512  # query chunks of 512 per batch = 4
HB = RPC // B    # rows per core per batch = 256

f32 = mybir.dt.float32
bf16 = mybir.dt.bfloat16

INV_SQRT_HD = 1.0 / float(np.sqrt(HD))
Act = mybir.ActivationFunctionType

_CACHED_NC = None


def build_nc(dbg=False):
    _pend.clear()
    nc = bacc.Bacc("TRN2", target_bir_lowering=False, debug=False)

    # x pre-shuffled on host so each [P, 512] tile is one contiguous DMA:
    # xP[fc, rc, p, c] = x.T[fc*128 + p, rc*512 + c]
    xP = nc.dram_tensor("xP", [FC, RT // 512, P, 512], bf16,
                        kind="ExternalInput")
    # projection weights pre-shuffled on host into SBUF layout
    # [p, fc, c] = W.T[fc*128 + p, c] so each loads as one contiguous DMA
    wqP = nc.dram_tensor("wqP", [P, FC, DPC], bf16, kind="ExternalInput")
    wkP = nc.dram_tensor("wkP", [P, FC, DPC], bf16, kind="ExternalInput")
    wvP = nc.dram_tensor("wvP", [P, FC, DPC], bf16, kind="ExternalInput")
    bq = nc.dram_tensor("bq", [P, HPC], f32, kind="ExternalInput")
    bk = nc.dram_tensor("bk", [P, HPC], f32, kind="ExternalInput")
    woT = nc.dram_tensor("woT", [D, D], bf16, kind="ExternalInput")
    # out rows: [0:256] = this core's batch-0 rows, [256:512] = batch-1 rows
    out = nc.dram_tensor("out", [RPC, D], f32, kind="ExternalOutput")
    if dbg:
        d_qT = nc.dram_tensor("d_qT", [P, HPC, RT], bf16, kind="ExternalOutput")
        d_kT = nc.dram_tensor("d_kT", [P, HPC, RT], bf16, kind="ExternalOutput")
        d_v = nc.dram_tensor("d_v", [P, RT // P, DPC], bf16,
                             kind="ExternalOutput")
        d_ctxl = nc.dram_tensor("d_ctxl", [HPC, B, P, W, HB], bf16,
                                kind="ExternalOutput")

    with tile.TileContext(nc) as tc:
        with (
            tc.tile_pool(name="persist", bufs=1) as persist,
            tc.tile_pool(name="dram", bufs=1, space="DRAM") as dram,
            tc.tile_pool(name="ps", bufs=2, space="PSUM") as ps,
            tc.tile_pool(name="wo", bufs=1) as wop,
            tc.tile_pool(name="xtp", bufs=12) as xtp,
            tc.tile_pool(name="wproj", bufs=1) as wproj,
            tc.tile_pool(name="attn_sb", bufs=4) as attn_sb,
            tc.tile_pool(name="norm_sb", bufs=2) as norm_sb,
            tc.tile_pool(name="ctxlp", bufs=1) as ctxlp,
            tc.tile_pool(name="osbp", bufs=2) as osbp,
        ):
            # ---- persistent SBUF state ----
            qT_sb = persist.tile([P, HPC, RT], bf16)      # [hd, h, row]
            kT_sb = persist.tile([P, HPC, RT], bf16)
            v_sb = persist.tile([P, RT // P, DPC], bf16)  # [row%128, rchunk, d]
            bq_sb = persist.tile([P, HPC], f32)
            bk_sb = persist.tile([P, HPC], f32)
            ones_sb = persist.tile([P, P], bf16)

            # one A2A per (head, batch): shard j = ctx.T for batch-b rows
            # [HB*j, HB*(j+1)) in head h's feature block
            a2a_in = [[dram.tile([W, HD, HB], bf16, name=f"a2a_in{h}{b}")
                       for b in range(B)] for h in range(HPC)]
            a2a_out = [[dram.tile([W, HD, HB], bf16, name=f"a2a_out{h}{b}")
                        for b in range(B)] for h in range(HPC)]

            nc.vector.memset(ones_sb[:], 1.0)
            nc.sync.dma_start(bq_sb[:], bq.ap())
            nc.sync.dma_start(bk_sb[:], bk.ap())

            # ---- HAM warmup: ~3us of tiny matmuls so the PE clock gate
            # opens before the real work arrives ----
            wtile = ps.tile([1, 4], f32, tag="cs", name="warm")
            for i in range(100):
                nc.tensor.matmul(wtile[:], ones_sb[:, 0:1],
                                 ones_sb[:, 0:4], start=True, stop=True)

            # ---- projection weights first, then full-Wo prefetch, all on
            # the scalar engine's DMA queue so the sync queue carries only
            # xt tiles during phase 1 ----
            wq_sb = wproj.tile([P, FC, DPC], bf16)
            wk_sb = wproj.tile([P, FC, DPC], bf16)
            wv_sb = wproj.tile([P, FC, DPC], bf16)
            # 4 chunks per weight so the first fc chunks land early
            for c4 in range(4):
                fsl = slice(4 * c4, 4 * c4 + 4)
                nc.scalar.dma_start(wq_sb[:, fsl, :], wqP.ap()[:, fsl, :])
                nc.scalar.dma_start(wk_sb[:, fsl, :], wkP.ap()[:, fsl, :])
                nc.scalar.dma_start(wv_sb[:, fsl, :], wvP.ap()[:, fsl, :])
            wo_tiles = {}

            def prefetch_wo(jc, hh):
                # 8 tiles per call, trickled through phase 1 on the idle
                # gpsimd queue so they never compete with phase-1 loads
                for i in range(W):
                    t = wop.tile([P, 512], bf16, name=f"wo_{jc}_{hh}_{i}")
                    nc.gpsimd.dma_start(
                        t[:],
                        woT.ap()[i * DPC + hh * HD:i * DPC + (hh + 1) * HD,
                                 jc * 512:(jc + 1) * 512])
                    wo_tiles[(jc, hh, i)] = t

            # ---- phase 1: QKV projections ----
            for rc in range(RT // 512):  # 8 row chunks of 512
                q_ps = ps.tile([P, 2, 512], f32, tag="stA", name="q_ps")
                k_ps = ps.tile([P, 2, 512], f32, tag="stA", name="k_ps")
                # one PSUM bank per V stream: matmul accumulation state is
                # bank-granular, so streams must not share a bank
                v_ps = [ps.tile([P, DPC], f32, tag=("ctx", "cs")[s // 2],
                                name=f"v_ps{s}")[:] for s in range(4)]
                for fc in range(FC):
                    xt = xtp.tile([P, 512], bf16, tag="xt")
                    nc.sync.dma_start(xt[:], xP.ap()[fc, rc])
                    st = fc == 0
                    sp = fc == FC - 1
                    # interleave short-stream V matmuls between long Q/K
                    # streams so each LDWEIGHTS hides behind a stream
                    for i in range(HPC):
                        nc.tensor.matmul(
                            q_ps[:, i, :], wq_sb[:, fc, i * HD:(i + 1) * HD],
                            xt[:], start=st, stop=sp)
                        nc.tensor.matmul(
                            v_ps[2 * i],
                            xt[:, 2 * i * P:(2 * i + 1) * P],
                            wv_sb[:, fc, :], start=st, stop=sp)
                        nc.tensor.matmul(
                            k_ps[:, i, :], wk_sb[:, fc, i * HD:(i + 1) * HD],
                            xt[:], start=st, stop=sp)
                        nc.tensor.matmul(
                            v_ps[2 * i + 1],
                            xt[:, (2 * i + 1) * P:(2 * i + 2) * P],
                            wv_sb[:, fc, :], start=st, stop=sp)
                # PSUM -> SBUF; Q/K on ACT (with bias), V on DVE
                for i in range(HPC):
                    nc.scalar.activation(
                        qT_sb[:, i, rc * 512:(rc + 1) * 512], q_ps[:, i, :],
                        Act.Identity, bias=bq_sb[:, i:i + 1])
                    nc.scalar.activation(
                        kT_sb[:, i, rc * 512:(rc + 1) * 512], k_ps[:, i, :],
                        Act.Identity, bias=bk_sb[:, i:i + 1])
                for s4 in range(4):
                    nc.vector.tensor_copy(v_sb[:, rc * 4 + s4, :], v_ps[s4])
                prefetch_wo(rc // 2, rc % 2)

            # ctxl staging tiles (consumed by phase 3); loaded right after
            # each quarter's collective on the gpsimd queue
            ctxl = [[ctxlp.tile([P, W, HB], bf16, name=f"ctxl{h}{b}")
                     for b in range(B)] for h in range(HPC)]

            # ---- phase 2: attention; one A2A per (h, b) quarter ----
            # PV + denominator matmuls run one kc-pair behind the score
            # matmuls so the PE never waits on the ACT exp; the pipeline
            # spans qc and quarter boundaries.
            prev = None

            def emit_pv(state):
                pt2, h, b, ctx_ps, cs_ps, kp = state
                for j in range(2):
                    kc = 2 * kp + j
                    st = kc == 0
                    sp = kc == KRC - 1
                    nc.tensor.matmul(
                        ctx_ps[:], v_sb[:, b * KRC + kc, h * HD:(h + 1) * HD],
                        pt2[:, j, :], start=st, stop=sp)
                    nc.tensor.matmul(
                        cs_ps[:], ones_sb[:], pt2[:, j, :], start=st, stop=sp)

            for h in range(HPC):
                for b in range(B):
                    for qc in range(QRC):
                        ctx_ps = ps.tile([P, 512], f32, tag="ctx",
                                         name="ctx_ps")
                        cs_ps = ps.tile([P, 512], f32, tag="cs", name="cs_ps")
                        for kp in range(KP):
                            st2 = ps.tile([P, 2, 512], f32, tag="stA",
                                          name="st2")
                            for j in range(2):
                                kc = 2 * kp + j
                                nc.tensor.matmul(
                                    st2[:, j, :],
                                    kT_sb[:, h, b * SEQ + kc * P:
                                          b * SEQ + (kc + 1) * P],
                                    qT_sb[:, h, b * SEQ + qc * 512:
                                          b * SEQ + (qc + 1) * 512],
                                    start=True, stop=True)
                            pt2 = attn_sb.tile([P, 2, 512], bf16, tag="pt")
                            nc.scalar.activation(pt2[:], st2[:], Act.Exp,
                                                 scale=INV_SQRT_HD)
                            if prev is not None:
                                emit_pv(prev)
                            prev = (pt2, h, b, ctx_ps, cs_ps, kp)
                        # normalize + ship the previous qc once its last
                        # PV/denominator matmuls have been emitted
                        if qc > 0:
                            _norm_ship(nc, norm_sb, a2a_in, *_pend.pop(0))
                        _pend.append((h, b, qc, ctx_ps, cs_ps))
                    # flush so this quarter's A2A can fire
                    emit_pv(prev)
                    prev = None
                    _norm_ship(nc, norm_sb, a2a_in, *_pend.pop(0))
                    nc.gpsimd.collective_compute(
                        "AllToAll", mybir.AluOpType.bypass,
                        replica_groups=[list(range(W))],
                        ins=[a2a_in[h][b][:]], outs=[a2a_out[h][b][:]])
                    for i in range(W):
                        nc.gpsimd.dma_start(
                            ctxl[h][b][:, i, :], a2a_out[h][b][i, :, :])

            if dbg:
                nc.sync.dma_start(d_qT.ap(), qT_sb[:])
                nc.sync.dma_start(d_kT.ap(), kT_sb[:])
                nc.sync.dma_start(d_v.ap(), v_sb[:])
                for hh in range(HPC):
                    for bb in range(B):
                        nc.sync.dma_start(d_ctxl.ap()[hh, bb],
                                          ctxl[hh][bb][:])

            # ---- phase 3: output projection ----
            # batch 0 first: its matmuls hide the last A2A (h=1, b=1)
            for bb in range(B):
                for jc in range(D // 512):
                    o_ps = [ps.tile([P, 512], f32, tag=("ctx", "cs")[r2],
                                    name=f"o_ps{r2}") for r2 in range(HB // P)]
                    for hh in range(HPC):
                        for i in range(W):
                            st = hh == 0 and i == 0
                            sp = hh == HPC - 1 and i == W - 1
                            for r2 in range(HB // P):
                                nc.tensor.matmul(
                                    o_ps[r2][:],
                                    ctxl[hh][bb][:, i, r2 * P:(r2 + 1) * P],
                                    wo_tiles[(jc, hh, i)][:],
                                    start=st, stop=sp)
                    for r2 in range(HB // P):
                        o_sb = osbp.tile([P, 512], f32, tag="osb")
                        nc.scalar.activation(o_sb[:], o_ps[r2][:], Act.Copy)
                        nc.sync.dma_start(
                            out.ap()[(bb * 2 + r2) * P:(bb * 2 + r2 + 1) * P,
                                     jc * 512:(jc + 1) * 512],
                            o_sb[:])

    nc.compile()
    return nc


_pend = []


def _norm_ship(nc, norm_sb, a2a_in, h, b, qc, ctx_ps, cs_ps):
    """Reciprocal of the pre-broadcast row sums, normalize ctx, ship to the
    A2A staging slots. Runs on DVE straight off PSUM."""
    rcp = norm_sb.tile([P, 512], f32, tag="rcp", name="rcp")
    nc.vector.reciprocal(rcp[:], cs_ps[:])
    ctxn = norm_sb.tile([P, 512], bf16, tag="ctxn", name="ctxn")
    nc.vector.tensor_mul(ctxn[:], ctx_ps[:], rcp[:])
    for s2 in range(2):
        nc.gpsimd.dma_start(a2a_in[h][b][2 * qc + s2, :, :],
                            ctxn[:, s2 * HB:(s2 + 1) * HB])


def kernel(x, Wq, bq, Wk, bk, Wv, bv, Wo, bo, _run_kwargs=None):
    global _CACHED_NC
    if _CACHED_NC is None:
        _CACHED_NC = build_nc()
    nc = _CACHED_NC

    bf = ml_dtypes.bfloat16
    x = np.asarray(x, dtype=np.float32)
    Wq = np.asarray(Wq, dtype=np.float32)
    Wk = np.asarray(Wk, dtype=np.float32)
    Wv = np.asarray(Wv, dtype=np.float32)
    Wo = np.asarray(Wo, dtype=np.float32)
    bq = np.asarray(bq, dtype=np.float32)
    bk = np.asarray(bk, dtype=np.float32)
    bv = np.asarray(bv, dtype=np.float32)
    bo = np.asarray(bo, dtype=np.float32)

    xT = x.reshape(RT, D).T.astype(bf)                         # [D, RT]
    xP = np.ascontiguousarray(
        xT.reshape(FC, P, RT // 512, 512).transpose(0, 2, 1, 3))
    woT = np.ascontiguousarray(Wo.T).astype(bf)                # [D, D]
    bo_eff = (bo + Wo @ bv).astype(np.float32)                 # [D]

    in_maps = []
    for i in range(W):
        sl = slice(i * DPC, (i + 1) * DPC)
        def shuf(Wm):
            # [P, FC, DPC]: [p, fc, c] = Wm[sl].T[fc*128 + p, c]
            return np.ascontiguousarray(
                Wm[sl, :].T.reshape(FC, P, DPC).transpose(1, 0, 2)).astype(bf)
        in_maps.append({
            "xP": xP,
            "wqP": shuf(Wq),
            "wkP": shuf(Wk),
            "wvP": shuf(Wv),
            "bq": np.ascontiguousarray(bq[sl].reshape(HPC, P).T),
            "bk": np.ascontiguousarray(bk[sl].reshape(HPC, P).T),
            "woT": woT,
        })

    kw = _run_kwargs or {}
    res = run_bass_kernel_spmd(nc, in_maps, core_ids=list(range(W)), **kw)

    full = np.empty((RT, D), dtype=np.float32)
    for i in range(W):
        o = res.results[i]["out"]
        full[i * HB:(i + 1) * HB, :] = o[:HB]              # batch 0 rows
        full[SEQ + i * HB:SEQ + (i + 1) * HB, :] = o[HB:]  # batch 1 rows
    full += bo_eff[None, :]
    out = full.reshape(B, SEQ, D)
    if kw:
        kernel.last_results = res
    return out
